# revision 46
# baseline (speedup 1.0000x reference)
"""Causal single-head attention on 8 TRN2 NeuronCores.

Problem: x[B=8,T=2048,E=1024] fp32, per-head Q/K/V projections (D=128) +
causal softmax attention. Sharding: data-parallel over batch B — one batch
element per core; Q/K/V weights replicated.

Per-core algorithm (matmuls in fp16 — 1 cycle/row on the PE, fp32 PSUM
accumulation; inputs pre-cast to fp16 on host so matmuls are exact given
the quantized inputs):

  x is fed pre-transposed (fp16) from host as xT [E, T] so the contraction
  dim (E) lies on SBUF partitions.
  1. qT/kT/vT [D, Tt] = sum_e W_chunk-as-stationary @ xT_chunk (PSUM accum
     over 8 e-chunks, N=512), bias added during the PSUM->SBUF move (DVE
     tensor_scalar_add, fp16 out).
  2. v natural [s, D] chunks by PE-transposing vT 128x128 blocks, stored
     ones-augmented: v_aug [s, 129] with col 128 == 1.
  3. Scores TRANSPOSED: S^T tile [s-chunk 128, t 512] = kT_slice @ qT.
     exp via ACT (scale=1/sqrt(D)), fp16 out into a per-t-tile ex_all
     buffer.  No max-subtraction needed (|scores| <~ 3).  Causality: s-chunks
     above the diagonal are skipped outright; diagonal chunks compute only
     the column range later read by PV, and a single 128x128 lower-triangle
     0/1 multiply masks the diagonal sub-block.
  4. PV in natural orientation per 128-row t-chunk: out_psum [t 128, 129]
     += ex_slice-as-stationary @ v_aug_chunk.  Column 128 accumulates the
     softmax denominator via the ones column.  No P transposes (ex_slice
     [s, t] IS P^T), no output transposes.
  5. Normalize: DVE reciprocal of column 128, tensor_scalar_mul rows,
     DMA out 64KB contiguous per t-chunk.
"""

import numpy as np

B, T, E, D = 8, 2048, 1024, 128
NT = 512                 # t-tile width (PSUM bank = 512 fp32)
N_TT = T // NT           # 4 t-tiles
N_TC = NT // 128         # 4 t-chunks per t-tile
N_EC = E // 128          # 8 e-chunks
N_SC = T // 128          # 16 s-chunks
VS = 132                 # v_aug free stride (129 used)
SCALE = float(1.0 / np.sqrt(D))

_cache: dict = {}


def _build(causal: bool):
    from contextlib import ExitStack
    import concourse.bass as bass
    import concourse.tile as tile
    from concourse import bacc, mybir
    from concourse.masks import make_identity

    f32 = mybir.dt.float32
    f16 = mybir.dt.float16
    AF = mybir.ActivationFunctionType

    nc = bacc.Bacc("TRN2", target_bir_lowering=False, debug=False,
                   num_devices=B)
    xT = nc.dram_tensor("xT", (E, T), f16, kind="ExternalInput").ap()
    Ws = {p: nc.dram_tensor(f"W{p}", (E, D), f16, kind="ExternalInput").ap()
          for p in "qkv"}
    bs = {p: nc.dram_tensor(f"b{p}", (D, 1), f32, kind="ExternalInput").ap()
          for p in "qkv"}
    out = nc.dram_tensor("out", (T, D), f32, kind="ExternalOutput").ap()

    with tile.TileContext(nc) as tc, ExitStack() as ctx:
        consts = ctx.enter_context(tc.tile_pool(name="consts", bufs=1))
        xt_pool = ctx.enter_context(tc.tile_pool(name="xt", bufs=3))
        qT_pool = ctx.enter_context(tc.tile_pool(name="qT", bufs=2))
        vT_pool = ctx.enter_context(tc.tile_pool(name="vT", bufs=2))
        ex_pool = ctx.enter_context(tc.tile_pool(name="ex", bufs=2))
        small = ctx.enter_context(tc.tile_pool(name="small", bufs=8))
        outp = ctx.enter_context(tc.tile_pool(name="outp", bufs=4))
        ps_qkv = ctx.enter_context(tc.tile_pool(name="ps_qkv", bufs=2,
                                                space="PSUM"))
        ps_s = ctx.enter_context(tc.tile_pool(name="ps_s", bufs=3,
                                              space="PSUM"))
        ps_o = ctx.enter_context(tc.tile_pool(name="ps_o", bufs=2,
                                              space="PSUM"))
        ps_t = ctx.enter_context(tc.tile_pool(name="ps_t", bufs=1,
                                              space="PSUM"))

        # ---- constants ----
        # Critical-path first: the very first projection matmul needs only
        # Wq chunk 0 (32KB) + xT chunk 0 (128KB); land those before the bulk.
        xt0 = xt_pool.tile([128, N_EC * NT], f16, tag="xt")
        wq_t = consts.tile([128, N_EC * 128], f16, tag="wq")
        w_t = {"q": wq_t}
        nc.sync.dma_start(w_t["q"][:, 0:128],
                          Ws["q"][0:128, :])
        nc.sync.dma_start(xt0[:, 0:NT], xT[0:128, 0:NT])
        nc.sync.dma_start(
            w_t["q"][:, 128:].rearrange("p (c d) -> p c d", c=N_EC - 1),
            Ws["q"][128:, :].rearrange("(c p) d -> p c d", p=128))
        nc.sync.dma_start(
            xt0[:, NT:].rearrange("p (c n) -> p c n", c=N_EC - 1),
            xT[128:, 0:NT].rearrange("(c p) n -> p c n", p=128))
        for p in "kv":
            wt = consts.tile([128, N_EC * 128], f16, tag=f"w{p}")
            nc.sync.dma_start(
                wt[:].rearrange("p (c d) -> p c d", c=N_EC),
                Ws[p].rearrange("(c p) d -> p c d", p=128))
            w_t[p] = wt
        b_t = {}
        for p in "qkv":
            bt = consts.tile([128, 1], f32, tag=f"b{p}")
            nc.sync.dma_start(bt[:], bs[p])
            b_t[p] = bt

        ident_h = consts.tile([128, 128], f16, tag="ident_h")
        make_identity(nc, ident_h[:])

        # PE warmup while input DMAs land: keeps HAM at K=8/8 so the first
        # real matmuls run at 2.4 GHz instead of 1.2.
        warm_t = consts.tile([128, 128], f16, tag="warm_t")
        nc.vector.memset(warm_t[:], 0.0)
        for _ in range(8):
            pw = ps_t.tile([128, 128], f32, tag="ps_t")
            nc.tensor.matmul(pw[:], warm_t[:], warm_t[:],
                             start=True, stop=True)

        masks_h = None
        if causal:
            # single lower-triangular (keep t>=s) 128x128 block
            masks_h = consts.tile([128, 128], f16, tag="masks_h")
            nc.gpsimd.memset(masks_h[:], 1.0)
            nc.gpsimd.affine_select(
                out=masks_h[:], in_=masks_h[:],
                compare_op=mybir.AluOpType.is_ge,
                fill=0.0, base=0, channel_multiplier=-1,
                pattern=[[1, 128]])

        kT_all = consts.tile([128, T], f16, tag="kT_all")
        v_all = consts.tile([128, N_SC * VS], f16, tag="v_all")
        nc.vector.memset(v_all[:], 1.0)  # keeps the ones column at VS*i+128

        def load_xt(jj):
            t0 = jj * NT
            xt = xt_pool.tile([128, N_EC * NT], f16, tag="xt")
            nc.sync.dma_start(
                xt[:].rearrange("p (c n) -> p c n", c=N_EC),
                xT[:, t0:t0 + NT].rearrange("(c p) n -> p c n", p=128))
            return xt

        qT_all = None
        if not causal:
            # full attention needs every t-tile's q resident before phase 2
            qT_all = consts.tile([128, T], f16, tag="qT_all")

        def proj(p, xt, dest):
            ps = ps_qkv.tile([128, NT], f32, tag="ps_qkv")
            for c in range(N_EC):
                nc.tensor.matmul(
                    ps[:], w_t[p][:, c * 128:(c + 1) * 128],
                    xt[:, c * NT:(c + 1) * NT],
                    start=(c == 0), stop=(c == N_EC - 1))
            nc.vector.tensor_scalar_add(dest, ps[:], b_t[p][:])

        def scores_exp(j, qT, ex_all):
            # Diagonal s-chunk m: columns t_local < 128*m are never read by
            # PV (those t-chunks exclude this s-chunk), so compute only
            # [128*m:NT] and mask just the 128-wide diagonal sub-block.
            n_sc = (j + 1) * N_TC if causal else N_SC
            for i in range(n_sc):
                m = i - j * N_TC
                off = 128 * m if (causal and m > 0) else 0
                ps = ps_s.tile([128, NT], f32, tag="ps_s")
                nc.tensor.matmul(ps[:, off:NT],
                                 kT_all[:, i * 128:(i + 1) * 128],
                                 qT[:, off:NT], start=True, stop=True)
                ex = ex_all[:, i * NT + off:(i + 1) * NT]
                nc.scalar.activation(ex, ps[:, off:NT], AF.Exp, scale=SCALE)
                if causal and m >= 0:
                    nc.vector.tensor_mul(
                        ex_all[:, i * NT + off:i * NT + off + 128],
                        ex_all[:, i * NT + off:i * NT + off + 128],
                        masks_h[:])

        def v_proj_transpose(j, xt):
            vT = vT_pool.tile([128, NT], f16, tag="vT")
            proj("v", xt, vT[:])
            for tch in range(N_TC):
                sc = j * N_TC + tch
                pt = ps_t.tile([128, 256], f16, tag="ps_t")
                nc.tensor.transpose(pt[:, 0:128],
                                    vT[:, tch * 128:(tch + 1) * 128],
                                    ident_h[:])
                nc.vector.tensor_copy(v_all[:, sc * VS:sc * VS + 128],
                                      pt[:, 0:128])

        def pv_out(j, ex_all):
            # PV natural per t-chunk; denominator rides in column 128
            t0 = j * NT
            ot = outp.tile([128, N_TC * 128], f32, tag="ot")
            for tch in range(N_TC):
                tc_glob = j * N_TC + tch
                n_i = tc_glob + 1 if causal else N_SC
                po = ps_o.tile([128, VS], f32, tag="ps_o")
                for i in range(n_i):
                    nc.tensor.matmul(
                        po[:, 0:129],
                        ex_all[:, i * NT + tch * 128:i * NT + (tch + 1) * 128],
                        v_all[:, i * VS:i * VS + 129],
                        start=(i == 0), stop=(i == n_i - 1),
                        skip_group_check=True)
                rec = small.tile([128, 1], f32, tag="rec")
                nc.vector.reciprocal(rec[:], po[:, 128:129])
                nc.vector.tensor_scalar_mul(
                    ot[:, tch * 128:(tch + 1) * 128], po[:, 0:128], rec[:])
                r0 = t0 + tch * 128
                nc.sync.dma_start(out[r0:r0 + 128, :],
                                  ot[:, tch * 128:(tch + 1) * 128])

        xt_tiles = {0: xt0}
        if causal:
            prev = None
            for j in range(N_TT):
                t0 = j * NT
                xt = xt_tiles.pop(j)
                qT = qT_pool.tile([128, NT], f16, tag="qT")
                proj("q", xt, qT[:])
                proj("k", xt, kT_all[:, t0:t0 + NT])
                ex_all = ex_pool.tile([128, N_SC * NT], f16, tag="ex")
                scores_exp(j, qT, ex_all)
                v_proj_transpose(j, xt)
                if j + 1 < N_TT:
                    xt_tiles[j + 1] = load_xt(j + 1)
                # PV runs one tile behind: the in-order PE then fills this
                # tile's exp-chain wait with the next tile's projections
                # instead of stalling on PV's last-chunk dependency.
                if prev is not None:
                    pv_out(*prev)
                prev = (j, ex_all)
            pv_out(*prev)
        else:
            # phase 1: all projections; phase 2: attention per t-tile
            for j in range(N_TT):
                t0 = j * NT
                xt = xt_tiles.pop(j)
                proj("q", xt, qT_all[:, t0:t0 + NT])
                proj("k", xt, kT_all[:, t0:t0 + NT])
                v_proj_transpose(j, xt)
                if j + 1 < N_TT:
                    xt_tiles[j + 1] = load_xt(j + 1)
            for j in range(N_TT):
                t0 = j * NT
                ex_all = ex_pool.tile([128, N_SC * NT], f16, tag="ex")
                scores_exp(j, qT_all[:, t0:t0 + NT], ex_all)
                pv_out(j, ex_all)

    nc.compile()
    return nc


def _get(causal: bool):
    if causal not in _cache:
        _cache[causal] = _build(causal)
    return _cache[causal]


def _make_in_maps(x, Wq, bq, Wk, bk, Wv, bv):
    x = np.asarray(x, dtype=np.float32)
    in_maps = []
    Wq16 = np.asarray(Wq, np.float16)
    Wk16 = np.asarray(Wk, np.float16)
    Wv16 = np.asarray(Wv, np.float16)
    bq_c = np.ascontiguousarray(np.asarray(bq, np.float32).reshape(D, 1))
    bk_c = np.ascontiguousarray(np.asarray(bk, np.float32).reshape(D, 1))
    bv_c = np.ascontiguousarray(np.asarray(bv, np.float32).reshape(D, 1))
    for b in range(B):
        in_maps.append({
            "xT": np.ascontiguousarray(x[b].T.astype(np.float16)),
            "Wq": Wq16, "Wk": Wk16, "Wv": Wv16,
            "bq": bq_c, "bk": bk_c, "bv": bv_c,
        })
    return in_maps


def kernel(x, Wq, bq, Wk, bk, Wv, bv, mask, **_ignored):
    from concourse.bass_utils import run_bass_kernel_spmd

    causal = bool(np.asarray(mask).item()) if mask is not None else False
    nc = _get(causal)
    in_maps = _make_in_maps(x, Wq, bq, Wk, bk, Wv, bv)
    res = run_bass_kernel_spmd(nc, in_maps, core_ids=list(range(B)))
    return np.stack([res.results[b]["out"] for b in range(B)], axis=0)


# revision 47
# speedup vs baseline: 1.0110x; 1.0110x over previous
"""Causal single-head attention on 8 TRN2 NeuronCores.

Problem: x[B=8,T=2048,E=1024] fp32, per-head Q/K/V projections (D=128) +
causal softmax attention. Sharding: data-parallel over batch B — one batch
element per core; Q/K/V weights replicated.

Per-core algorithm (matmuls in fp16 — 1 cycle/row on the PE, fp32 PSUM
accumulation; inputs pre-cast to fp16 on host so matmuls are exact given
the quantized inputs):

  x is fed pre-transposed (fp16) from host as xT [E, T] so the contraction
  dim (E) lies on SBUF partitions.
  1. qT/kT/vT [D, Tt] = sum_e W_chunk-as-stationary @ xT_chunk (PSUM accum
     over 8 e-chunks, N=512), bias added during the PSUM->SBUF move (DVE
     tensor_scalar_add, fp16 out).
  2. v natural [s, D] chunks by PE-transposing vT 128x128 blocks, stored
     ones-augmented: v_aug [s, 129] with col 128 == 1.
  3. Scores TRANSPOSED: S^T tile [s-chunk 128, t 512] = kT_slice @ qT.
     exp via ACT (scale=1/sqrt(D)), fp16 out into a per-t-tile ex_all
     buffer.  No max-subtraction needed (|scores| <~ 3).  Causality: s-chunks
     above the diagonal are skipped outright; diagonal chunks compute only
     the column range later read by PV, and a single 128x128 lower-triangle
     0/1 multiply masks the diagonal sub-block.
  4. PV in natural orientation per 128-row t-chunk: out_psum [t 128, 129]
     += ex_slice-as-stationary @ v_aug_chunk.  Column 128 accumulates the
     softmax denominator via the ones column.  No P transposes (ex_slice
     [s, t] IS P^T), no output transposes.
  5. Normalize: DVE reciprocal of column 128, tensor_scalar_mul rows,
     DMA out 64KB contiguous per t-chunk.
"""

import numpy as np

B, T, E, D = 8, 2048, 1024, 128
NT = 512                 # t-tile width (PSUM bank = 512 fp32)
N_TT = T // NT           # 4 t-tiles
N_TC = NT // 128         # 4 t-chunks per t-tile
N_EC = E // 128          # 8 e-chunks
N_SC = T // 128          # 16 s-chunks
VS = 132                 # v_aug free stride (129 used)
SCALE = float(1.0 / np.sqrt(D))

_cache: dict = {}


def _build(causal: bool):
    from contextlib import ExitStack
    import concourse.bass as bass
    import concourse.tile as tile
    from concourse import bacc, mybir
    from concourse.masks import make_identity

    f32 = mybir.dt.float32
    f16 = mybir.dt.float16
    AF = mybir.ActivationFunctionType

    nc = bacc.Bacc("TRN2", target_bir_lowering=False, debug=False,
                   num_devices=B)
    xT = nc.dram_tensor("xT", (E, T), f16, kind="ExternalInput").ap()
    Ws = {p: nc.dram_tensor(f"W{p}", (E, D), f16, kind="ExternalInput").ap()
          for p in "qkv"}
    bs = {p: nc.dram_tensor(f"b{p}", (D, 1), f32, kind="ExternalInput").ap()
          for p in "qkv"}
    out = nc.dram_tensor("out", (T, D), f32, kind="ExternalOutput").ap()

    with tile.TileContext(nc) as tc, ExitStack() as ctx:
        consts = ctx.enter_context(tc.tile_pool(name="consts", bufs=1))
        xt_pool = ctx.enter_context(tc.tile_pool(name="xt", bufs=3))
        qT_pool = ctx.enter_context(tc.tile_pool(name="qT", bufs=2))
        vT_pool = ctx.enter_context(tc.tile_pool(name="vT", bufs=2))
        ex_pool = ctx.enter_context(tc.tile_pool(name="ex", bufs=2))
        small = ctx.enter_context(tc.tile_pool(name="small", bufs=8))
        outp = ctx.enter_context(tc.tile_pool(name="outp", bufs=4))
        ps_qkv = ctx.enter_context(tc.tile_pool(name="ps_qkv", bufs=2,
                                                space="PSUM"))
        ps_s = ctx.enter_context(tc.tile_pool(name="ps_s", bufs=3,
                                              space="PSUM"))
        ps_o = ctx.enter_context(tc.tile_pool(name="ps_o", bufs=2,
                                              space="PSUM"))
        ps_t = ctx.enter_context(tc.tile_pool(name="ps_t", bufs=1,
                                              space="PSUM"))

        # ---- constants ----
        xt0 = xt_pool.tile([128, N_EC * NT], f16, tag="xt")
        nc.sync.dma_start(
            xt0[:, 0:2 * NT].rearrange("p (c n) -> p c n", c=2),
            xT[0:256, 0:NT].rearrange("(c p) n -> p c n", p=128))
        w_t = {}
        for p in "qkv":
            wt = consts.tile([128, N_EC * 128], f16, tag=f"w{p}")
            nc.sync.dma_start(
                wt[:].rearrange("p (c d) -> p c d", c=N_EC),
                Ws[p].rearrange("(c p) d -> p c d", p=128))
            w_t[p] = wt
            if p == "q":
                nc.sync.dma_start(
                    xt0[:, 2 * NT:].rearrange("p (c n) -> p c n", c=N_EC - 2),
                    xT[256:, 0:NT].rearrange("(c p) n -> p c n", p=128))
        b_t = {}
        for p in "qkv":
            bt = consts.tile([128, 1], f32, tag=f"b{p}")
            nc.sync.dma_start(bt[:], bs[p])
            b_t[p] = bt

        ident_h = consts.tile([128, 128], f16, tag="ident_h")
        make_identity(nc, ident_h[:])

        # PE warmup while input DMAs land: keeps HAM at K=8/8 so the first
        # real matmuls run at 2.4 GHz instead of 1.2.
        warm_t = consts.tile([128, 128], f16, tag="warm_t")
        nc.vector.memset(warm_t[:], 0.0)
        for _ in range(12):
            pw = ps_t.tile([128, 128], f32, tag="ps_t")
            nc.tensor.matmul(pw[:], warm_t[:], warm_t[:],
                             start=True, stop=True)

        masks_h = None
        if causal:
            # single lower-triangular (keep t>=s) 128x128 block
            masks_h = consts.tile([128, 128], f16, tag="masks_h")
            nc.gpsimd.memset(masks_h[:], 1.0)
            nc.gpsimd.affine_select(
                out=masks_h[:], in_=masks_h[:],
                compare_op=mybir.AluOpType.is_ge,
                fill=0.0, base=0, channel_multiplier=-1,
                pattern=[[1, 128]])

        kT_all = consts.tile([128, T], f16, tag="kT_all")
        v_all = consts.tile([128, N_SC * VS], f16, tag="v_all")
        nc.vector.memset(v_all[:], 1.0)  # keeps the ones column at VS*i+128

        def load_xt(jj):
            t0 = jj * NT
            xt = xt_pool.tile([128, N_EC * NT], f16, tag="xt")
            nc.sync.dma_start(
                xt[:].rearrange("p (c n) -> p c n", c=N_EC),
                xT[:, t0:t0 + NT].rearrange("(c p) n -> p c n", p=128))
            return xt

        qT_all = None
        if not causal:
            # full attention needs every t-tile's q resident before phase 2
            qT_all = consts.tile([128, T], f16, tag="qT_all")

        def proj(p, xt, dest):
            ps = ps_qkv.tile([128, NT], f32, tag="ps_qkv")
            for c in range(N_EC):
                nc.tensor.matmul(
                    ps[:], w_t[p][:, c * 128:(c + 1) * 128],
                    xt[:, c * NT:(c + 1) * NT],
                    start=(c == 0), stop=(c == N_EC - 1))
            nc.vector.tensor_scalar_add(dest, ps[:], b_t[p][:])

        def scores_exp(j, qT, ex_all):
            # Diagonal s-chunk m: columns t_local < 128*m are never read by
            # PV (those t-chunks exclude this s-chunk), so compute only
            # [128*m:NT] and mask just the 128-wide diagonal sub-block.
            n_sc = (j + 1) * N_TC if causal else N_SC
            for i in range(n_sc):
                m = i - j * N_TC
                off = 128 * m if (causal and m > 0) else 0
                ps = ps_s.tile([128, NT], f32, tag="ps_s")
                nc.tensor.matmul(ps[:, off:NT],
                                 kT_all[:, i * 128:(i + 1) * 128],
                                 qT[:, off:NT], start=True, stop=True)
                ex = ex_all[:, i * NT + off:(i + 1) * NT]
                nc.scalar.activation(ex, ps[:, off:NT], AF.Exp, scale=SCALE)
                if causal and m >= 0:
                    nc.vector.tensor_mul(
                        ex_all[:, i * NT + off:i * NT + off + 128],
                        ex_all[:, i * NT + off:i * NT + off + 128],
                        masks_h[:])

        def v_proj_transpose(j, xt):
            vT = vT_pool.tile([128, NT], f16, tag="vT")
            proj("v", xt, vT[:])
            for tch in range(N_TC):
                sc = j * N_TC + tch
                pt = ps_t.tile([128, 256], f16, tag="ps_t")
                nc.tensor.transpose(pt[:, 0:128],
                                    vT[:, tch * 128:(tch + 1) * 128],
                                    ident_h[:])
                nc.vector.tensor_copy(v_all[:, sc * VS:sc * VS + 128],
                                      pt[:, 0:128])

        def pv_out(j, ex_all):
            # PV natural per t-chunk; denominator rides in column 128
            t0 = j * NT
            ot = outp.tile([128, N_TC * 128], f32, tag="ot")
            for tch in range(N_TC):
                tc_glob = j * N_TC + tch
                n_i = tc_glob + 1 if causal else N_SC
                po = ps_o.tile([128, VS], f32, tag="ps_o")
                for i in range(n_i):
                    nc.tensor.matmul(
                        po[:, 0:129],
                        ex_all[:, i * NT + tch * 128:i * NT + (tch + 1) * 128],
                        v_all[:, i * VS:i * VS + 129],
                        start=(i == 0), stop=(i == n_i - 1),
                        skip_group_check=True)
                rec = small.tile([128, 1], f32, tag="rec")
                nc.vector.reciprocal(rec[:], po[:, 128:129])
                nc.vector.tensor_scalar_mul(
                    ot[:, tch * 128:(tch + 1) * 128], po[:, 0:128], rec[:])
                r0 = t0 + tch * 128
                nc.sync.dma_start(out[r0:r0 + 128, :],
                                  ot[:, tch * 128:(tch + 1) * 128])

        xt_tiles = {0: xt0}
        if causal:
            prev = None
            for j in range(N_TT):
                t0 = j * NT
                xt = xt_tiles.pop(j)
                qT = qT_pool.tile([128, NT], f16, tag="qT")
                proj("q", xt, qT[:])
                proj("k", xt, kT_all[:, t0:t0 + NT])
                ex_all = ex_pool.tile([128, N_SC * NT], f16, tag="ex")
                scores_exp(j, qT, ex_all)
                v_proj_transpose(j, xt)
                if j + 1 < N_TT:
                    xt_tiles[j + 1] = load_xt(j + 1)
                # PV runs one tile behind: the in-order PE then fills this
                # tile's exp-chain wait with the next tile's projections
                # instead of stalling on PV's last-chunk dependency.
                if prev is not None:
                    pv_out(*prev)
                prev = (j, ex_all)
            pv_out(*prev)
        else:
            # phase 1: all projections; phase 2: attention per t-tile
            for j in range(N_TT):
                t0 = j * NT
                xt = xt_tiles.pop(j)
                proj("q", xt, qT_all[:, t0:t0 + NT])
                proj("k", xt, kT_all[:, t0:t0 + NT])
                v_proj_transpose(j, xt)
                if j + 1 < N_TT:
                    xt_tiles[j + 1] = load_xt(j + 1)
            for j in range(N_TT):
                t0 = j * NT
                ex_all = ex_pool.tile([128, N_SC * NT], f16, tag="ex")
                scores_exp(j, qT_all[:, t0:t0 + NT], ex_all)
                pv_out(j, ex_all)

    nc.compile()
    return nc


def _get(causal: bool):
    if causal not in _cache:
        _cache[causal] = _build(causal)
    return _cache[causal]


def _make_in_maps(x, Wq, bq, Wk, bk, Wv, bv):
    x = np.asarray(x, dtype=np.float32)
    in_maps = []
    Wq16 = np.asarray(Wq, np.float16)
    Wk16 = np.asarray(Wk, np.float16)
    Wv16 = np.asarray(Wv, np.float16)
    bq_c = np.ascontiguousarray(np.asarray(bq, np.float32).reshape(D, 1))
    bk_c = np.ascontiguousarray(np.asarray(bk, np.float32).reshape(D, 1))
    bv_c = np.ascontiguousarray(np.asarray(bv, np.float32).reshape(D, 1))
    for b in range(B):
        in_maps.append({
            "xT": np.ascontiguousarray(x[b].T.astype(np.float16)),
            "Wq": Wq16, "Wk": Wk16, "Wv": Wv16,
            "bq": bq_c, "bk": bk_c, "bv": bv_c,
        })
    return in_maps


def kernel(x, Wq, bq, Wk, bk, Wv, bv, mask, **_ignored):
    from concourse.bass_utils import run_bass_kernel_spmd

    causal = bool(np.asarray(mask).item()) if mask is not None else False
    nc = _get(causal)
    in_maps = _make_in_maps(x, Wq, bq, Wk, bk, Wv, bv)
    res = run_bass_kernel_spmd(nc, in_maps, core_ids=list(range(B)))
    return np.stack([res.results[b]["out"] for b in range(B)], axis=0)


# revision 48
# speedup vs baseline: 1.0197x; 1.0086x over previous
"""Causal single-head attention on 8 TRN2 NeuronCores.

Problem: x[B=8,T=2048,E=1024] fp32, per-head Q/K/V projections (D=128) +
causal softmax attention. Sharding: data-parallel over batch B — one batch
element per core; Q/K/V weights replicated.

Per-core algorithm (matmuls in fp16 — 1 cycle/row on the PE, fp32 PSUM
accumulation; inputs pre-cast to fp16 on host so matmuls are exact given
the quantized inputs):

  x is fed pre-transposed (fp16) from host as xT [E, T] so the contraction
  dim (E) lies on SBUF partitions.
  1. qT/kT/vT [D, Tt] = sum_e W_chunk-as-stationary @ xT_chunk (PSUM accum
     over 8 e-chunks, N=512), bias added during the PSUM->SBUF move (DVE
     tensor_scalar_add, fp16 out).
  2. v natural [s, D] chunks by PE-transposing vT 128x128 blocks, stored
     ones-augmented: v_aug [s, 129] with col 128 == 1.
  3. Scores TRANSPOSED: S^T tile [s-chunk 128, t 512] = kT_slice @ qT.
     exp via ACT (scale=1/sqrt(D)), fp16 out into a per-t-tile ex_all
     buffer.  No max-subtraction needed (|scores| <~ 3).  Causality: s-chunks
     above the diagonal are skipped outright; diagonal chunks compute only
     the column range later read by PV, and a single 128x128 lower-triangle
     0/1 multiply masks the diagonal sub-block.
  4. PV in natural orientation per 128-row t-chunk: out_psum [t 128, 129]
     += ex_slice-as-stationary @ v_aug_chunk.  Column 128 accumulates the
     softmax denominator via the ones column.  No P transposes (ex_slice
     [s, t] IS P^T), no output transposes.
  5. Normalize: DVE reciprocal of column 128, tensor_scalar_mul rows,
     DMA out 64KB contiguous per t-chunk.
"""

import numpy as np

B, T, E, D = 8, 2048, 1024, 128
NT = 512                 # t-tile width (PSUM bank = 512 fp32)
N_TT = T // NT           # 4 t-tiles
N_TC = NT // 128         # 4 t-chunks per t-tile
N_EC = E // 128          # 8 e-chunks
N_SC = T // 128          # 16 s-chunks
VS = 132                 # v_aug free stride (129 used)
SCALE = float(1.0 / np.sqrt(D))

_cache: dict = {}


def _build(causal: bool):
    from contextlib import ExitStack
    import concourse.bass as bass
    import concourse.tile as tile
    from concourse import bacc, mybir
    from concourse.masks import make_identity

    f32 = mybir.dt.float32
    f16 = mybir.dt.float16
    AF = mybir.ActivationFunctionType

    nc = bacc.Bacc("TRN2", target_bir_lowering=False, debug=False,
                   num_devices=B)
    xT = nc.dram_tensor("xT", (E, T), f16, kind="ExternalInput").ap()
    Ws = {p: nc.dram_tensor(f"W{p}", (E, D), f16, kind="ExternalInput").ap()
          for p in "qkv"}
    bs = {p: nc.dram_tensor(f"b{p}", (D, 1), f32, kind="ExternalInput").ap()
          for p in "qkv"}
    out = nc.dram_tensor("out", (T, D), f32, kind="ExternalOutput").ap()

    with tile.TileContext(nc) as tc, ExitStack() as ctx:
        consts = ctx.enter_context(tc.tile_pool(name="consts", bufs=1))
        xt_pool = ctx.enter_context(tc.tile_pool(name="xt", bufs=3))
        qT_pool = ctx.enter_context(tc.tile_pool(name="qT", bufs=2))
        vT_pool = ctx.enter_context(tc.tile_pool(name="vT", bufs=2))
        ex_pool = ctx.enter_context(tc.tile_pool(name="ex", bufs=3))
        small = ctx.enter_context(tc.tile_pool(name="small", bufs=8))
        outp = ctx.enter_context(tc.tile_pool(name="outp", bufs=4))
        ps_qkv = ctx.enter_context(tc.tile_pool(name="ps_qkv", bufs=2,
                                                space="PSUM"))
        ps_s = ctx.enter_context(tc.tile_pool(name="ps_s", bufs=3,
                                              space="PSUM"))
        ps_o = ctx.enter_context(tc.tile_pool(name="ps_o", bufs=2,
                                              space="PSUM"))
        ps_t = ctx.enter_context(tc.tile_pool(name="ps_t", bufs=1,
                                              space="PSUM"))

        # ---- constants ----
        xt0 = xt_pool.tile([128, N_EC * NT], f16, tag="xt")
        nc.sync.dma_start(
            xt0[:, 0:2 * NT].rearrange("p (c n) -> p c n", c=2),
            xT[0:256, 0:NT].rearrange("(c p) n -> p c n", p=128))
        w_t = {}
        for p in "qkv":
            wt = consts.tile([128, N_EC * 128], f16, tag=f"w{p}")
            nc.sync.dma_start(
                wt[:].rearrange("p (c d) -> p c d", c=N_EC),
                Ws[p].rearrange("(c p) d -> p c d", p=128))
            w_t[p] = wt
            if p == "q":
                nc.sync.dma_start(
                    xt0[:, 2 * NT:].rearrange("p (c n) -> p c n", c=N_EC - 2),
                    xT[256:, 0:NT].rearrange("(c p) n -> p c n", p=128))
        b_t = {}
        for p in "qkv":
            bt = consts.tile([128, 1], f32, tag=f"b{p}")
            nc.sync.dma_start(bt[:], bs[p])
            b_t[p] = bt

        ident_h = consts.tile([128, 128], f16, tag="ident_h")
        make_identity(nc, ident_h[:])

        # PE warmup while input DMAs land: keeps HAM at K=8/8 so the first
        # real matmuls run at 2.4 GHz instead of 1.2.
        warm_t = consts.tile([128, 128], f16, tag="warm_t")
        nc.vector.memset(warm_t[:], 0.0)
        for _ in range(12):
            pw = ps_t.tile([128, 128], f32, tag="ps_t")
            nc.tensor.matmul(pw[:], warm_t[:], warm_t[:],
                             start=True, stop=True)

        masks_h = None
        if causal:
            # single lower-triangular (keep t>=s) 128x128 block
            masks_h = consts.tile([128, 128], f16, tag="masks_h")
            nc.gpsimd.memset(masks_h[:], 1.0)
            nc.gpsimd.affine_select(
                out=masks_h[:], in_=masks_h[:],
                compare_op=mybir.AluOpType.is_ge,
                fill=0.0, base=0, channel_multiplier=-1,
                pattern=[[1, 128]])

        kT_all = consts.tile([128, T], f16, tag="kT_all")
        v_all = consts.tile([128, N_SC * VS], f16, tag="v_all")
        nc.vector.memset(v_all[:], 1.0)  # keeps the ones column at VS*i+128

        def load_xt(jj):
            t0 = jj * NT
            xt = xt_pool.tile([128, N_EC * NT], f16, tag="xt")
            nc.sync.dma_start(
                xt[:].rearrange("p (c n) -> p c n", c=N_EC),
                xT[:, t0:t0 + NT].rearrange("(c p) n -> p c n", p=128))
            return xt

        qT_all = None
        if not causal:
            # full attention needs every t-tile's q resident before phase 2
            qT_all = consts.tile([128, T], f16, tag="qT_all")

        def proj(p, xt, dest):
            ps = ps_qkv.tile([128, NT], f32, tag="ps_qkv")
            for c in range(N_EC):
                nc.tensor.matmul(
                    ps[:], w_t[p][:, c * 128:(c + 1) * 128],
                    xt[:, c * NT:(c + 1) * NT],
                    start=(c == 0), stop=(c == N_EC - 1))
            nc.vector.tensor_scalar_add(dest, ps[:], b_t[p][:])

        def scores_exp(j, qT, ex_all):
            # Diagonal s-chunk m: columns t_local < 128*m are never read by
            # PV (those t-chunks exclude this s-chunk), so compute only
            # [128*m:NT] and mask just the 128-wide diagonal sub-block.
            n_sc = (j + 1) * N_TC if causal else N_SC
            for i in range(n_sc):
                m = i - j * N_TC
                off = 128 * m if (causal and m > 0) else 0
                ps = ps_s.tile([128, NT], f32, tag="ps_s")
                nc.tensor.matmul(ps[:, off:NT],
                                 kT_all[:, i * 128:(i + 1) * 128],
                                 qT[:, off:NT], start=True, stop=True)
                ex = ex_all[:, i * NT + off:(i + 1) * NT]
                nc.scalar.activation(ex, ps[:, off:NT], AF.Exp, scale=SCALE)
                if causal and m >= 0:
                    nc.vector.tensor_mul(
                        ex_all[:, i * NT + off:i * NT + off + 128],
                        ex_all[:, i * NT + off:i * NT + off + 128],
                        masks_h[:])

        def v_proj_transpose(j, xt):
            vT = vT_pool.tile([128, NT], f16, tag="vT")
            proj("v", xt, vT[:])
            for tch in range(N_TC):
                sc = j * N_TC + tch
                pt = ps_t.tile([128, 256], f16, tag="ps_t")
                nc.tensor.transpose(pt[:, 0:128],
                                    vT[:, tch * 128:(tch + 1) * 128],
                                    ident_h[:])
                nc.vector.tensor_copy(v_all[:, sc * VS:sc * VS + 128],
                                      pt[:, 0:128])

        def pv_out(j, ex_all):
            # PV natural per t-chunk; denominator rides in column 128
            t0 = j * NT
            ot = outp.tile([128, N_TC * 128], f32, tag="ot")
            for tch in range(N_TC):
                tc_glob = j * N_TC + tch
                n_i = tc_glob + 1 if causal else N_SC
                po = ps_o.tile([128, VS], f32, tag="ps_o")
                for i in range(n_i):
                    nc.tensor.matmul(
                        po[:, 0:129],
                        ex_all[:, i * NT + tch * 128:i * NT + (tch + 1) * 128],
                        v_all[:, i * VS:i * VS + 129],
                        start=(i == 0), stop=(i == n_i - 1),
                        skip_group_check=True)
                rec = small.tile([128, 1], f32, tag="rec")
                nc.vector.reciprocal(rec[:], po[:, 128:129])
                nc.vector.tensor_scalar_mul(
                    ot[:, tch * 128:(tch + 1) * 128], po[:, 0:128], rec[:])
                r0 = t0 + tch * 128
                nc.sync.dma_start(out[r0:r0 + 128, :],
                                  ot[:, tch * 128:(tch + 1) * 128])

        xt_tiles = {0: xt0}
        if causal:
            prev = None
            for j in range(N_TT):
                t0 = j * NT
                xt = xt_tiles.pop(j)
                qT = qT_pool.tile([128, NT], f16, tag="qT")
                proj("q", xt, qT[:])
                proj("k", xt, kT_all[:, t0:t0 + NT])
                ex_all = ex_pool.tile([128, N_SC * NT], f16, tag="ex")
                scores_exp(j, qT, ex_all)
                v_proj_transpose(j, xt)
                if j + 1 < N_TT:
                    xt_tiles[j + 1] = load_xt(j + 1)
                # PV runs one tile behind: the in-order PE then fills this
                # tile's exp-chain wait with the next tile's projections
                # instead of stalling on PV's last-chunk dependency.
                if prev is not None:
                    pv_out(*prev)
                prev = (j, ex_all)
            pv_out(*prev)
        else:
            # phase 1: all projections; phase 2: attention per t-tile
            for j in range(N_TT):
                t0 = j * NT
                xt = xt_tiles.pop(j)
                proj("q", xt, qT_all[:, t0:t0 + NT])
                proj("k", xt, kT_all[:, t0:t0 + NT])
                v_proj_transpose(j, xt)
                if j + 1 < N_TT:
                    xt_tiles[j + 1] = load_xt(j + 1)
            for j in range(N_TT):
                t0 = j * NT
                ex_all = ex_pool.tile([128, N_SC * NT], f16, tag="ex")
                scores_exp(j, qT_all[:, t0:t0 + NT], ex_all)
                pv_out(j, ex_all)

    nc.compile()
    return nc


def _get(causal: bool):
    if causal not in _cache:
        _cache[causal] = _build(causal)
    return _cache[causal]


def _make_in_maps(x, Wq, bq, Wk, bk, Wv, bv):
    x = np.asarray(x, dtype=np.float32)
    in_maps = []
    Wq16 = np.asarray(Wq, np.float16)
    Wk16 = np.asarray(Wk, np.float16)
    Wv16 = np.asarray(Wv, np.float16)
    bq_c = np.ascontiguousarray(np.asarray(bq, np.float32).reshape(D, 1))
    bk_c = np.ascontiguousarray(np.asarray(bk, np.float32).reshape(D, 1))
    bv_c = np.ascontiguousarray(np.asarray(bv, np.float32).reshape(D, 1))
    for b in range(B):
        in_maps.append({
            "xT": np.ascontiguousarray(x[b].T.astype(np.float16)),
            "Wq": Wq16, "Wk": Wk16, "Wv": Wv16,
            "bq": bq_c, "bk": bk_c, "bv": bv_c,
        })
    return in_maps


def kernel(x, Wq, bq, Wk, bk, Wv, bv, mask, **_ignored):
    from concourse.bass_utils import run_bass_kernel_spmd

    causal = bool(np.asarray(mask).item()) if mask is not None else False
    nc = _get(causal)
    in_maps = _make_in_maps(x, Wq, bq, Wk, bk, Wv, bv)
    res = run_bass_kernel_spmd(nc, in_maps, core_ids=list(range(B)))
    return np.stack([res.results[b]["out"] for b in range(B)], axis=0)
